# revision 25
# baseline (speedup 1.0000x reference)
"""Block-ELL sparse linear on 8 Trainium2 cores — bf16 + fp8-DoubleRow hybrid.

Same dense-ified data-parallel strategy as v2, but NK8 of the 32
contraction k-tiles run as fp8-e4m3 DoubleRow matmuls (2 k-tiles per MM at
~0.5 cycles/row) and accumulate into the same PSUM group as the bf16
k-tiles. Weight/x fp8 scales (x/S, W*S) cancel in the product, so no
descale pass is needed. S=8 keeps the small W values (std 0.02) out of
e4m3 subnormals. Measured end-to-end rel err (exact bit-level host-side
simulation over the full output): 1.67e-2 vs the 2e-2 budget.

DR quads are interleaved between the last bf16 quads so the 256-column
(no-FWL) DR LDWEIGHTS hide behind bf16 matmul streams.
"""

import numpy as np
import ml_dtypes
from contextlib import ExitStack

import concourse.bass as bass
import concourse.bacc as bacc
import concourse.tile as tile
from concourse import mybir
from concourse.bass_utils import run_bass_kernel_spmd

N_TOK = 8192
R = 256
C = 256
K = 64
B = 16
D_IN = C * B   # 4096
D_OUT = R * B  # 4096
NCORES = 8
TOK = N_TOK // NCORES  # 1024
TT = 2
TOKT = TOK // TT       # 512

NK8 = 6          # k-tiles (of 32) on the fp8 DoubleRow path; must be even.
                 # 8 would be ~10us faster (measured 417.5us) but its
                 # scale-relative max err is 2.47e-2 > the 2e-2 gate; 6 keeps
                 # both metrics under (L2 1.667e-2, max 1.891e-2).
SW = 8.0         # fp8 weight scale (x gets 1/SW)

_cache = {}


def _build_program(feat_tiles: int, nk8: int):
    key = (feat_tiles, nk8)
    if key in _cache:
        return _cache[key]

    bf16 = mybir.dt.bfloat16
    f32 = mybir.dt.float32
    f8 = mybir.dt.float8e4
    DR = mybir.MatmulPerfMode.DoubleRow

    FEAT = feat_tiles          # total contraction tiles of 128
    NBF = FEAT - nk8           # bf16 k-tiles
    NPR = nk8 // 2             # fp8 DR pairs
    OGB = 16
    OG2 = 2

    nc = bacc.Bacc("TRN2", target_bir_lowering=False, debug=False,
                   num_devices=NCORES)

    xT_d = nc.dram_tensor("xT", [NBF * 128, TOK], bf16, kind="ExternalInput").ap()
    w_d = nc.dram_tensor("W", [OGB, NBF, 128, 256], bf16, kind="ExternalInput").ap()
    if NPR:
        x8_d = nc.dram_tensor("x8", [128, NPR, 2, TOK], f8,
                              kind="ExternalInput").ap()
        w8_d = nc.dram_tensor("W8", [OGB, 128, NPR, 2, 256], f8,
                              kind="ExternalInput").ap()
    yT_d = nc.dram_tensor("yT", [D_OUT // 128, 128, TOK], bf16,
                          kind="ExternalOutput").ap()

    with tile.TileContext(nc) as tc, ExitStack() as ctx:
        xpool = ctx.enter_context(tc.tile_pool(name="x", bufs=1))
        # 20-deep W ring: once a W tile runs late, slot recycling couples
        # issue to consumption and the full DMA latency is exposed per tile;
        # deeper lookahead lets the prefetch rebuild after HBM-contention
        # hiccups (trace showed LDWEIGHTS waiting 1.4-2.9us on W DMAs)
        wpool = ctx.enter_context(tc.tile_pool(name="w", bufs=20))

        w8pool = ctx.enter_context(tc.tile_pool(name="w8", bufs=3))
        ppool = ctx.enter_context(tc.tile_pool(name="ps", bufs=2, space="PSUM"))
        ypool = ctx.enter_context(tc.tile_pool(name="y", bufs=3))

        # resident bf16 x^T [128, NBF*TOK]
        xt = xpool.tile([128, NBF * TOK], bf16)
        xT_r = xT_d.rearrange("(f p) n -> f p n", p=128)
        for f in range(NBF):
            if f == 0:
                # chunk 0 lands in two halves: the t=0 matmuls can start
                # ~1.3us earlier on the first half while the second streams
                for h in range(TT):
                    nc.sync.dma_start(
                        out=xt[:, h * TOKT:(h + 1) * TOKT],
                        in_=xT_r[0][:, h * TOKT:(h + 1) * TOKT])
            else:
                nc.sync.dma_start(out=xt[:, f * TOK:(f + 1) * TOK], in_=xT_r[f])
        if NPR:
            # resident fp8 x (needed only from the end of the first pass on,
            # so it queues after the bf16 chunks)
            x8t = xpool.tile([128, NPR * 2 * TOK], f8)
            nc.sync.dma_start(out=x8t[:], in_=x8_d.rearrange("p j i n -> p (j i n)"))
            x8v = x8t.rearrange("p (j i n) -> p j i n", j=NPR, i=2)

        def bf_quad(ps, wt, f, first):
            for og2 in range(OG2):
                for t in range(TT):
                    nc.tensor.matmul(
                        ps[og2 * TT + t][:],
                        wt[:, og2 * 128:(og2 + 1) * 128],
                        xt[:, f * TOK + t * TOKT: f * TOK + (t + 1) * TOKT],
                        start=first, stop=False,
                    )

        def dr_quad(ps, w8v, j, last):
            for og2 in range(OG2):
                for t in range(TT):
                    nc.tensor.matmul(
                        ps[og2 * TT + t][:],
                        w8v[:, j, :, og2 * 128:(og2 + 1) * 128],
                        x8v[:, j, :, t * TOKT:(t + 1) * TOKT],
                        start=False, stop=last,
                        perf_mode=DR,
                    )

        def og_pass(ps, ogb, wts=None):
            """One out-group's full contraction: NBF bf16 quads with the DR
            quads woven into the tail so DR LDWEIGHTS overlap bf16 streams."""
            if NPR:
                w8t = w8pool.tile([128, NPR * 2 * 256], f8,
                                  name=f"w8t_{ogb}", tag="w8t")
                nc.scalar.dma_start(out=w8t[:],
                                    in_=w8_d[ogb].rearrange("p j i m -> p (j i m)"))
                w8v = w8t.rearrange("p (j i m) -> p j i m", j=NPR, i=2)
            else:
                w8v = None
            # schedule: bf16 f = 0..NBF-1 with a DR quad after each of the
            # last NPR bf16 quads
            dr_after = {NBF - NPR + j: j for j in range(NPR)}
            for f in range(NBF):
                if wts is not None:
                    wt = wts[f]
                else:
                    wt = wpool.tile([128, 256], bf16, name=f"wt_{ogb}_{f}", tag="wt")
                    nc.scalar.dma_start(out=wt[:], in_=w_d[ogb, f])
                bf_quad(ps, wt, f, first=(f == 0))
                j = dr_after.get(f)
                if j is not None:
                    dr_quad(ps, w8v, j, last=(j == NPR - 1))

        def evict(ps, ogb, last):
            for og2 in range(OG2):
                yt = ypool.tile([128, TOK], bf16, name=f"yt_{ogb}_{og2}", tag="yt")
                og = ogb * OG2 + og2
                for t in range(TT):
                    if (og2 * TT + t) % 2 == 0:
                        nc.vector.tensor_copy(yt[:, t * TOKT:(t + 1) * TOKT],
                                              ps[og2 * TT + t][:])
                    else:
                        nc.scalar.copy(yt[:, t * TOKT:(t + 1) * TOKT],
                                       ps[og2 * TT + t][:])
                    eng = nc.scalar if (last and t == 1) else nc.sync
                    eng.dma_start(
                        out=yT_d[og, :, t * TOKT:(t + 1) * TOKT],
                        in_=yt[:, t * TOKT:(t + 1) * TOKT])

        # --- phase 1: ogb 0,1 interleaved per-f (PE chases the x stream)
        ps01 = []
        for g in range(2):
            ps01.append([ppool.tile([128, TOKT], f32, name=f"ps_{g}_{i}",
                                    tag=f"ps{i}") for i in range(4)])
        w8vs = []
        for f in range(NBF):
            wts = []
            for g in range(2):
                wt = wpool.tile([128, 256], bf16, name=f"wt_{g}_{f}", tag="wt")
                nc.scalar.dma_start(out=wt[:], in_=w_d[g, f])
                wts.append(wt)
            if f == 1 and NPR:
                # fp8 weights for the pair, queued behind the first bf16 tiles
                for g in range(2):
                    w8t = w8pool.tile([128, NPR * 2 * 256], f8,
                                      name=f"w8t_{g}", tag="w8t")
                    nc.scalar.dma_start(out=w8t[:],
                                        in_=w8_d[g].rearrange("p j i m -> p (j i m)"))
                    w8vs.append(w8t.rearrange("p (j i m) -> p j i m", j=NPR, i=2))
            if f == 0:
                # t-major so the first 8 matmuls need only chunk 0's first
                # half (start=True per psum tile is preserved: one f=0 MM each)
                for t in range(TT):
                    for g in range(2):
                        for og2 in range(OG2):
                            nc.tensor.matmul(
                                ps01[g][og2 * TT + t][:],
                                wts[g][:, og2 * 128:(og2 + 1) * 128],
                                xt[:, t * TOKT:(t + 1) * TOKT],
                                start=True, stop=False,
                            )
            else:
                for g in range(2):
                    bf_quad(ps01[g], wts[g], f, first=False)
        # phase-1 DR quads run after the whole bf16 sweep: by then the
        # x8 DMA (queued last on the sync ring, ~45us) has long landed
        for j in range(NPR):
            for g in range(2):
                dr_quad(ps01[g], w8vs[g], j, last=(j == NPR - 1))
        for g in range(2):
            evict(ps01[g], g, last=False)

        # --- phase 2: remaining out-groups, pipelined
        for ogb in range(2, OGB):
            ps = [ppool.tile([128, TOKT], f32, name=f"ps_{ogb}_{i}", tag=f"ps{i}")
                  for i in range(4)]
            og_pass(ps, ogb)
            evict(ps, ogb, last=(ogb == OGB - 1))

    nc.compile()
    _cache[key] = nc
    return nc


def _scatter_dense(values: np.ndarray, col_indices: np.ndarray) -> np.ndarray:
    Wd = np.zeros((C, B, R, B), np.float32)  # [c, i, r, o]
    vT = np.ascontiguousarray(values.transpose(0, 1, 3, 2))  # [r, k, i, o]
    for r in range(R):
        np.add.at(Wd[:, :, r, :], (col_indices[r],), vT[r])
    return Wd.reshape(D_IN, D_OUT)


def _run(x, values, bias, col_indices, trace=False):
    x = np.asarray(x, np.float32)
    values = np.asarray(values, np.float32)
    bias = np.asarray(bias, np.float32)
    col_indices = np.asarray(col_indices, np.int32)

    W = _scatter_dense(values, col_indices)
    has_bias = bool(np.any(bias))
    FEAT = D_IN // 128 + (1 if has_bias else 0)

    xT = np.ascontiguousarray(x.T)  # [D_IN, N_TOK]
    if has_bias:
        xT = np.concatenate([xT, np.zeros((128, N_TOK), np.float32)], 0)
        xT[D_IN, :] = 1.0
        W = np.concatenate([W, np.zeros((128, D_OUT), np.float32)], 0)
        W[D_IN, :] = bias

    nk8 = NK8
    kf = nk8 * 128
    e4 = ml_dtypes.float8_e4m3
    bf = ml_dtypes.bfloat16

    # fp8 head of the contraction
    x8 = np.ascontiguousarray(
        (xT[:kf] / SW).reshape(nk8 // 2, 2, 128, N_TOK).transpose(2, 0, 1, 3)
    ).astype(e4)                                # [128, NPR, 2, N_TOK]
    W8 = np.ascontiguousarray(
        (W[:kf] * SW).reshape(nk8 // 2, 2, 128, 16, 256).transpose(3, 2, 0, 1, 4)
    ).astype(e4)                                # [OGB, 128, NPR, 2, 256]

    # bf16 tail
    NBF = FEAT - nk8
    Wt = np.ascontiguousarray(
        W[kf:].reshape(NBF, 128, 16, 256).transpose(2, 0, 1, 3)).astype(bf)
    xTc = xT[kf:].astype(bf)

    nc = _build_program(FEAT, nk8)

    in_maps = []
    for c in range(NCORES):
        m = {
            "xT": np.ascontiguousarray(xTc[:, c * TOK:(c + 1) * TOK]),
            "W": Wt,
        }
        if nk8:
            m["x8"] = np.ascontiguousarray(x8[:, :, :, c * TOK:(c + 1) * TOK])
            m["W8"] = W8
        in_maps.append(m)

    res = run_bass_kernel_spmd(nc, in_maps, list(range(NCORES)), trace=trace)

    y = np.empty((N_TOK, D_OUT), np.float32)
    for c in range(NCORES):
        yT = res.results[c]["yT"].reshape(D_OUT, TOK).astype(np.float32)
        y[c * TOK:(c + 1) * TOK, :] = yT.T
    return y, res


def kernel(x: np.ndarray, values: np.ndarray, bias: np.ndarray,
           col_indices: np.ndarray) -> np.ndarray:
    y, _ = _run(x, values, bias, col_indices)
    if not np.isfinite(y).all() or np.abs(y).max() > 1e6:
        y, _ = _run(x, values, bias, col_indices)
    return y


def run_traced(x, values, bias, col_indices):
    return _run(x, values, bias, col_indices, trace=True)[1]


# revision 26
# speedup vs baseline: 1.0007x; 1.0007x over previous
"""Block-ELL sparse linear on 8 Trainium2 cores — bf16 + fp8-DoubleRow hybrid.

Same dense-ified data-parallel strategy as v2, but NK8 of the 32
contraction k-tiles run as fp8-e4m3 DoubleRow matmuls (2 k-tiles per MM at
~0.5 cycles/row) and accumulate into the same PSUM group as the bf16
k-tiles. Weight/x fp8 scales (x/S, W*S) cancel in the product, so no
descale pass is needed. S=8 keeps the small W values (std 0.02) out of
e4m3 subnormals. Measured end-to-end rel err (exact bit-level host-side
simulation over the full output): 1.67e-2 vs the 2e-2 budget.

DR quads are interleaved between the last bf16 quads so the 256-column
(no-FWL) DR LDWEIGHTS hide behind bf16 matmul streams.
"""

import numpy as np
import ml_dtypes
from contextlib import ExitStack

import concourse.bass as bass
import concourse.bacc as bacc
import concourse.tile as tile
from concourse import mybir
from concourse.bass_utils import run_bass_kernel_spmd

N_TOK = 8192
R = 256
C = 256
K = 64
B = 16
D_IN = C * B   # 4096
D_OUT = R * B  # 4096
NCORES = 8
TOK = N_TOK // NCORES  # 1024
TT = 2
TOKT = TOK // TT       # 512

NK8 = 6          # k-tiles (of 32) on the fp8 DoubleRow path; must be even.
                 # 8 would be ~10us faster (measured 417.5us) but its
                 # scale-relative max err is 2.47e-2 > the 2e-2 gate; 6 keeps
                 # both metrics under (L2 1.667e-2, max 1.891e-2).
SW = 8.0         # fp8 weight scale (x gets 1/SW)

_cache = {}


def _build_program(feat_tiles: int, nk8: int):
    key = (feat_tiles, nk8)
    if key in _cache:
        return _cache[key]

    bf16 = mybir.dt.bfloat16
    f32 = mybir.dt.float32
    f8 = mybir.dt.float8e4
    DR = mybir.MatmulPerfMode.DoubleRow

    FEAT = feat_tiles          # total contraction tiles of 128
    NBF = FEAT - nk8           # bf16 k-tiles
    NPR = nk8 // 2             # fp8 DR pairs
    OGB = 16
    OG2 = 2

    nc = bacc.Bacc("TRN2", target_bir_lowering=False, debug=False,
                   num_devices=NCORES)

    xT_d = nc.dram_tensor("xT", [NBF * 128, TOK], bf16, kind="ExternalInput").ap()
    w_d = nc.dram_tensor("W", [OGB, NBF, 128, 256], bf16, kind="ExternalInput").ap()
    if NPR:
        x8_d = nc.dram_tensor("x8", [128, NPR, 2, TOK], f8,
                              kind="ExternalInput").ap()
        w8_d = nc.dram_tensor("W8", [OGB, 128, NPR, 2, 256], f8,
                              kind="ExternalInput").ap()
    yT_d = nc.dram_tensor("yT", [D_OUT // 128, 128, TOK], bf16,
                          kind="ExternalOutput").ap()

    with tile.TileContext(nc) as tc, ExitStack() as ctx:
        xpool = ctx.enter_context(tc.tile_pool(name="x", bufs=1))
        wpool = ctx.enter_context(tc.tile_pool(name="w", bufs=12))

        w8pool = ctx.enter_context(tc.tile_pool(name="w8", bufs=3))
        ppool = ctx.enter_context(tc.tile_pool(name="ps", bufs=2, space="PSUM"))
        ypool = ctx.enter_context(tc.tile_pool(name="y", bufs=3))

        # resident bf16 x^T [128, NBF*TOK]
        xt = xpool.tile([128, NBF * TOK], bf16)
        xT_r = xT_d.rearrange("(f p) n -> f p n", p=128)
        for f in range(NBF):
            if f == 0:
                # chunk 0 lands in two halves: the t=0 matmuls can start
                # ~1.3us earlier on the first half while the second streams
                for h in range(TT):
                    nc.sync.dma_start(
                        out=xt[:, h * TOKT:(h + 1) * TOKT],
                        in_=xT_r[0][:, h * TOKT:(h + 1) * TOKT])
            else:
                nc.sync.dma_start(out=xt[:, f * TOK:(f + 1) * TOK], in_=xT_r[f])
        if NPR:
            # resident fp8 x (needed only from the end of the first pass on,
            # so it queues after the bf16 chunks)
            x8t = xpool.tile([128, NPR * 2 * TOK], f8)
            nc.sync.dma_start(out=x8t[:], in_=x8_d.rearrange("p j i n -> p (j i n)"))
            x8v = x8t.rearrange("p (j i n) -> p j i n", j=NPR, i=2)

        def bf_quad(ps, wt, f, first):
            for og2 in range(OG2):
                for t in range(TT):
                    nc.tensor.matmul(
                        ps[og2 * TT + t][:],
                        wt[:, og2 * 128:(og2 + 1) * 128],
                        xt[:, f * TOK + t * TOKT: f * TOK + (t + 1) * TOKT],
                        start=first, stop=False,
                    )

        def dr_quad(ps, w8v, j, last):
            for og2 in range(OG2):
                for t in range(TT):
                    nc.tensor.matmul(
                        ps[og2 * TT + t][:],
                        w8v[:, j, :, og2 * 128:(og2 + 1) * 128],
                        x8v[:, j, :, t * TOKT:(t + 1) * TOKT],
                        start=False, stop=last,
                        perf_mode=DR,
                    )

        def og_pass(ps, ogb, wts=None):
            """One out-group's full contraction: NBF bf16 quads with the DR
            quads woven into the tail so DR LDWEIGHTS overlap bf16 streams."""
            if NPR:
                w8t = w8pool.tile([128, NPR * 2 * 256], f8,
                                  name=f"w8t_{ogb}", tag="w8t")
                nc.scalar.dma_start(out=w8t[:],
                                    in_=w8_d[ogb].rearrange("p j i m -> p (j i m)"))
                w8v = w8t.rearrange("p (j i m) -> p j i m", j=NPR, i=2)
            else:
                w8v = None
            # schedule: bf16 f = 0..NBF-1 with a DR quad after each of the
            # last NPR bf16 quads
            dr_after = {NBF - NPR + j: j for j in range(NPR)}
            for f in range(NBF):
                if wts is not None:
                    wt = wts[f]
                else:
                    wt = wpool.tile([128, 256], bf16, name=f"wt_{ogb}_{f}", tag="wt")
                    nc.scalar.dma_start(out=wt[:], in_=w_d[ogb, f])
                bf_quad(ps, wt, f, first=(f == 0))
                j = dr_after.get(f)
                if j is not None:
                    dr_quad(ps, w8v, j, last=(j == NPR - 1))

        def evict(ps, ogb, last):
            for og2 in range(OG2):
                yt = ypool.tile([128, TOK], bf16, name=f"yt_{ogb}_{og2}", tag="yt")
                og = ogb * OG2 + og2
                for t in range(TT):
                    if (og2 * TT + t) % 2 == 0:
                        nc.vector.tensor_copy(yt[:, t * TOKT:(t + 1) * TOKT],
                                              ps[og2 * TT + t][:])
                    else:
                        nc.scalar.copy(yt[:, t * TOKT:(t + 1) * TOKT],
                                       ps[og2 * TT + t][:])
                    eng = nc.scalar if (last and t == 1) else nc.sync
                    eng.dma_start(
                        out=yT_d[og, :, t * TOKT:(t + 1) * TOKT],
                        in_=yt[:, t * TOKT:(t + 1) * TOKT])

        # --- phase 1: ogb 0,1 interleaved per-f (PE chases the x stream)
        ps01 = []
        for g in range(2):
            ps01.append([ppool.tile([128, TOKT], f32, name=f"ps_{g}_{i}",
                                    tag=f"ps{i}") for i in range(4)])
        w8vs = []
        for f in range(NBF):
            wts = []
            for g in range(2):
                wt = wpool.tile([128, 256], bf16, name=f"wt_{g}_{f}", tag="wt")
                nc.scalar.dma_start(out=wt[:], in_=w_d[g, f])
                wts.append(wt)
            if f == 1 and NPR:
                # fp8 weights for the pair, queued behind the first bf16 tiles
                for g in range(2):
                    w8t = w8pool.tile([128, NPR * 2 * 256], f8,
                                      name=f"w8t_{g}", tag="w8t")
                    nc.scalar.dma_start(out=w8t[:],
                                        in_=w8_d[g].rearrange("p j i m -> p (j i m)"))
                    w8vs.append(w8t.rearrange("p (j i m) -> p j i m", j=NPR, i=2))
            if f == 0:
                # t-major so the first 8 matmuls need only chunk 0's first
                # half (start=True per psum tile is preserved: one f=0 MM each)
                for t in range(TT):
                    for g in range(2):
                        for og2 in range(OG2):
                            nc.tensor.matmul(
                                ps01[g][og2 * TT + t][:],
                                wts[g][:, og2 * 128:(og2 + 1) * 128],
                                xt[:, t * TOKT:(t + 1) * TOKT],
                                start=True, stop=False,
                            )
            else:
                for g in range(2):
                    bf_quad(ps01[g], wts[g], f, first=False)
        # phase-1 DR quads run after the whole bf16 sweep: by then the
        # x8 DMA (queued last on the sync ring, ~45us) has long landed
        for j in range(NPR):
            for g in range(2):
                dr_quad(ps01[g], w8vs[g], j, last=(j == NPR - 1))
        for g in range(2):
            evict(ps01[g], g, last=False)

        # --- phase 2: remaining out-groups, pipelined
        for ogb in range(2, OGB):
            ps = [ppool.tile([128, TOKT], f32, name=f"ps_{ogb}_{i}", tag=f"ps{i}")
                  for i in range(4)]
            og_pass(ps, ogb)
            evict(ps, ogb, last=(ogb == OGB - 1))

    nc.compile()
    _cache[key] = nc
    return nc


def _scatter_dense(values: np.ndarray, col_indices: np.ndarray) -> np.ndarray:
    Wd = np.zeros((C, B, R, B), np.float32)  # [c, i, r, o]
    vT = np.ascontiguousarray(values.transpose(0, 1, 3, 2))  # [r, k, i, o]
    for r in range(R):
        np.add.at(Wd[:, :, r, :], (col_indices[r],), vT[r])
    return Wd.reshape(D_IN, D_OUT)


def _run(x, values, bias, col_indices, trace=False):
    x = np.asarray(x, np.float32)
    values = np.asarray(values, np.float32)
    bias = np.asarray(bias, np.float32)
    col_indices = np.asarray(col_indices, np.int32)

    W = _scatter_dense(values, col_indices)
    has_bias = bool(np.any(bias))
    FEAT = D_IN // 128 + (1 if has_bias else 0)

    xT = np.ascontiguousarray(x.T)  # [D_IN, N_TOK]
    if has_bias:
        xT = np.concatenate([xT, np.zeros((128, N_TOK), np.float32)], 0)
        xT[D_IN, :] = 1.0
        W = np.concatenate([W, np.zeros((128, D_OUT), np.float32)], 0)
        W[D_IN, :] = bias

    nk8 = NK8
    kf = nk8 * 128
    e4 = ml_dtypes.float8_e4m3
    bf = ml_dtypes.bfloat16

    # fp8 head of the contraction
    x8 = np.ascontiguousarray(
        (xT[:kf] / SW).reshape(nk8 // 2, 2, 128, N_TOK).transpose(2, 0, 1, 3)
    ).astype(e4)                                # [128, NPR, 2, N_TOK]
    W8 = np.ascontiguousarray(
        (W[:kf] * SW).reshape(nk8 // 2, 2, 128, 16, 256).transpose(3, 2, 0, 1, 4)
    ).astype(e4)                                # [OGB, 128, NPR, 2, 256]

    # bf16 tail
    NBF = FEAT - nk8
    Wt = np.ascontiguousarray(
        W[kf:].reshape(NBF, 128, 16, 256).transpose(2, 0, 1, 3)).astype(bf)
    xTc = xT[kf:].astype(bf)

    nc = _build_program(FEAT, nk8)

    in_maps = []
    for c in range(NCORES):
        m = {
            "xT": np.ascontiguousarray(xTc[:, c * TOK:(c + 1) * TOK]),
            "W": Wt,
        }
        if nk8:
            m["x8"] = np.ascontiguousarray(x8[:, :, :, c * TOK:(c + 1) * TOK])
            m["W8"] = W8
        in_maps.append(m)

    res = run_bass_kernel_spmd(nc, in_maps, list(range(NCORES)), trace=trace)

    y = np.empty((N_TOK, D_OUT), np.float32)
    for c in range(NCORES):
        yT = res.results[c]["yT"].reshape(D_OUT, TOK).astype(np.float32)
        y[c * TOK:(c + 1) * TOK, :] = yT.T
    return y, res


def kernel(x: np.ndarray, values: np.ndarray, bias: np.ndarray,
           col_indices: np.ndarray) -> np.ndarray:
    y, _ = _run(x, values, bias, col_indices)
    if not np.isfinite(y).all() or np.abs(y).max() > 1e6:
        y, _ = _run(x, values, bias, col_indices)
    return y


def run_traced(x, values, bias, col_indices):
    return _run(x, values, bias, col_indices, trace=True)[1]


# revision 30
# speedup vs baseline: 1.0144x; 1.0137x over previous
"""Block-ELL sparse linear on 8 Trainium2 cores — bf16 + fp8-DoubleRow hybrid.

Same dense-ified data-parallel strategy as v2, but NK8 of the 32
contraction k-tiles run as fp8-e4m3 DoubleRow matmuls (2 k-tiles per MM at
~0.5 cycles/row) and accumulate into the same PSUM group as the bf16
k-tiles. Weight/x fp8 scales (x/S, W*S) cancel in the product, so no
descale pass is needed. S=8 keeps the small W values (std 0.02) out of
e4m3 subnormals. Measured end-to-end rel err (exact bit-level host-side
simulation over the full output): 1.67e-2 vs the 2e-2 budget.

DR quads are interleaved between the last bf16 quads so the 256-column
(no-FWL) DR LDWEIGHTS hide behind bf16 matmul streams.
"""

import numpy as np
import ml_dtypes
from contextlib import ExitStack

import concourse.bass as bass
import concourse.bacc as bacc
import concourse.tile as tile
from concourse import mybir
from concourse.bass_utils import run_bass_kernel_spmd

N_TOK = 8192
R = 256
C = 256
K = 64
B = 16
D_IN = C * B   # 4096
D_OUT = R * B  # 4096
NCORES = 8
TOK = N_TOK // NCORES  # 1024
TT = 2
TOKT = TOK // TT       # 512

NK8 = 6          # k-tiles (of 32) on the fp8 DoubleRow path; must be even.
                 # 8 would be ~10us faster (measured 417.5us) but its
                 # scale-relative max err is 2.47e-2 > the 2e-2 gate; 6 keeps
                 # both metrics under (L2 1.667e-2, max 1.891e-2).
SW = 8.0         # fp8 weight scale (x gets 1/SW)

_cache = {}


def _build_program(feat_tiles: int, nk8: int):
    key = (feat_tiles, nk8)
    if key in _cache:
        return _cache[key]

    bf16 = mybir.dt.bfloat16
    f32 = mybir.dt.float32
    f8 = mybir.dt.float8e4
    DR = mybir.MatmulPerfMode.DoubleRow

    FEAT = feat_tiles          # total contraction tiles of 128
    NBF = FEAT - nk8           # bf16 k-tiles
    NPR = nk8 // 2             # fp8 DR pairs
    OGB = 16
    OG2 = 2

    nc = bacc.Bacc("TRN2", target_bir_lowering=False, debug=False,
                   num_devices=NCORES)

    xT_d = nc.dram_tensor("xT", [NBF * 128, TOK], bf16, kind="ExternalInput").ap()
    w_d = nc.dram_tensor("W", [OGB, NBF, 128, 256], bf16, kind="ExternalInput").ap()
    if NPR:
        x8_d = nc.dram_tensor("x8", [128, NPR, 2, TOK], f8,
                              kind="ExternalInput").ap()
        w8_d = nc.dram_tensor("W8", [OGB, 128, NPR, 2, 256], f8,
                              kind="ExternalInput").ap()
    yT_d = nc.dram_tensor("yT", [D_OUT // 128, 128, TOK], bf16,
                          kind="ExternalOutput").ap()

    with tile.TileContext(nc) as tc, ExitStack() as ctx:
        xpool = ctx.enter_context(tc.tile_pool(name="x", bufs=1))
        # 6 bufs of f-pair (128KB) tiles = same prefetch byte-depth as the
        # previous 12x64KB, at half the DMA issue count
        wpool = ctx.enter_context(tc.tile_pool(name="w", bufs=6))

        w8pool = ctx.enter_context(tc.tile_pool(name="w8", bufs=3))
        ppool = ctx.enter_context(tc.tile_pool(name="ps", bufs=2, space="PSUM"))
        ypool = ctx.enter_context(tc.tile_pool(name="y", bufs=3))

        # resident bf16 x^T [128, NBF*TOK]
        xt = xpool.tile([128, NBF * TOK], bf16)
        xT_r = xT_d.rearrange("(f p) n -> f p n", p=128)
        for f in range(NBF):
            if f == 0:
                # chunk 0 lands in two halves: the t=0 matmuls can start
                # ~1.3us earlier on the first half while the second streams
                for h in range(TT):
                    nc.sync.dma_start(
                        out=xt[:, h * TOKT:(h + 1) * TOKT],
                        in_=xT_r[0][:, h * TOKT:(h + 1) * TOKT])
            else:
                nc.sync.dma_start(out=xt[:, f * TOK:(f + 1) * TOK], in_=xT_r[f])
        if NPR:
            # resident fp8 x (needed only from the end of the first pass on,
            # so it queues after the bf16 chunks)
            x8t = xpool.tile([128, NPR * 2 * TOK], f8)
            nc.sync.dma_start(out=x8t[:], in_=x8_d.rearrange("p j i n -> p (j i n)"))
            x8v = x8t.rearrange("p (j i n) -> p j i n", j=NPR, i=2)

        def bf_quad(ps, wt, f, first):
            for og2 in range(OG2):
                for t in range(TT):
                    nc.tensor.matmul(
                        ps[og2 * TT + t][:],
                        wt[:, og2 * 128:(og2 + 1) * 128],
                        xt[:, f * TOK + t * TOKT: f * TOK + (t + 1) * TOKT],
                        start=first, stop=False,
                    )

        def dr_quad(ps, w8v, j, last):
            for og2 in range(OG2):
                for t in range(TT):
                    nc.tensor.matmul(
                        ps[og2 * TT + t][:],
                        w8v[:, j, :, og2 * 128:(og2 + 1) * 128],
                        x8v[:, j, :, t * TOKT:(t + 1) * TOKT],
                        start=False, stop=last,
                        perf_mode=DR,
                    )

        def og_pass(ps, ogb, wts=None):
            """One out-group's full contraction: NBF bf16 quads with the DR
            quads woven into the tail so DR LDWEIGHTS overlap bf16 streams."""
            if NPR:
                w8t = w8pool.tile([128, NPR * 2 * 256], f8,
                                  name=f"w8t_{ogb}", tag="w8t")
                nc.scalar.dma_start(out=w8t[:],
                                    in_=w8_d[ogb].rearrange("p j i m -> p (j i m)"))
                w8v = w8t.rearrange("p (j i m) -> p j i m", j=NPR, i=2)
            else:
                w8v = None
            # schedule: bf16 f = 0..NBF-1 with a DR quad after each of the
            # last NPR bf16 quads. W arrives as f-pairs: one 128KB DMA per
            # two k-tiles (same prefetch bytes as before at half the bufs,
            # but half the issue count and supply-latency chain).
            dr_after = {NBF - NPR + j: j for j in range(NPR)}
            for fp in range(NBF // 2):
                wt2 = wpool.tile([128, 512], bf16, name=f"wt_{ogb}_{fp}", tag="wt")
                nc.scalar.dma_start(
                    out=wt2[:].rearrange("p (f c) -> p f c", f=2),
                    in_=w_d[ogb, 2 * fp:2 * fp + 2].rearrange("f p c -> p f c"))
                for fi in range(2):
                    f = 2 * fp + fi
                    bf_quad(ps, wt2[:, fi * 256:(fi + 1) * 256], f,
                            first=(f == 0))
                    j = dr_after.get(f)
                    if j is not None:
                        dr_quad(ps, w8v, j, last=(j == NPR - 1))

        def evict(ps, ogb, last):
            for og2 in range(OG2):
                yt = ypool.tile([128, TOK], bf16, name=f"yt_{ogb}_{og2}", tag="yt")
                og = ogb * OG2 + og2
                for t in range(TT):
                    if (og2 * TT + t) % 2 == 0:
                        nc.vector.tensor_copy(yt[:, t * TOKT:(t + 1) * TOKT],
                                              ps[og2 * TT + t][:])
                    else:
                        nc.scalar.copy(yt[:, t * TOKT:(t + 1) * TOKT],
                                       ps[og2 * TT + t][:])
                    eng = nc.scalar if (last and t == 1) else nc.sync
                    eng.dma_start(
                        out=yT_d[og, :, t * TOKT:(t + 1) * TOKT],
                        in_=yt[:, t * TOKT:(t + 1) * TOKT])

        # --- phase 1: ogb 0,1 interleaved per-f (PE chases the x stream)
        ps01 = []
        for g in range(2):
            ps01.append([ppool.tile([128, TOKT], f32, name=f"ps_{g}_{i}",
                                    tag=f"ps{i}") for i in range(4)])
        w8vs = []
        for f in range(NBF):
            # both out-groups' W tiles for this f in one 2-segment DMA
            wt2 = wpool.tile([128, 512], bf16, name=f"wt_p1_{f}", tag="wt")
            nc.scalar.dma_start(
                out=wt2[:].rearrange("p (g c) -> p g c", g=2),
                in_=w_d[0:2, f].rearrange("g p c -> p g c"))
            wts = [wt2[:, 0:256], wt2[:, 256:512]]
            if f == 1 and NPR:
                # fp8 weights for the pair, queued behind the first bf16 tiles
                for g in range(2):
                    w8t = w8pool.tile([128, NPR * 2 * 256], f8,
                                      name=f"w8t_{g}", tag="w8t")
                    nc.scalar.dma_start(out=w8t[:],
                                        in_=w8_d[g].rearrange("p j i m -> p (j i m)"))
                    w8vs.append(w8t.rearrange("p (j i m) -> p j i m", j=NPR, i=2))
            if f == 0:
                # t-major so the first 8 matmuls need only chunk 0's first
                # half (start=True per psum tile is preserved: one f=0 MM each)
                for t in range(TT):
                    for g in range(2):
                        for og2 in range(OG2):
                            nc.tensor.matmul(
                                ps01[g][og2 * TT + t][:],
                                wts[g][:, og2 * 128:(og2 + 1) * 128],
                                xt[:, t * TOKT:(t + 1) * TOKT],
                                start=True, stop=False,
                            )
            else:
                for g in range(2):
                    bf_quad(ps01[g], wts[g], f, first=False)
        # phase-1 DR quads run after the whole bf16 sweep: by then the
        # x8 DMA (queued last on the sync ring, ~45us) has long landed
        for j in range(NPR):
            for g in range(2):
                dr_quad(ps01[g], w8vs[g], j, last=(j == NPR - 1))
        for g in range(2):
            evict(ps01[g], g, last=False)

        # --- phase 2: remaining out-groups, pipelined
        for ogb in range(2, OGB):
            ps = [ppool.tile([128, TOKT], f32, name=f"ps_{ogb}_{i}", tag=f"ps{i}")
                  for i in range(4)]
            og_pass(ps, ogb)
            evict(ps, ogb, last=(ogb == OGB - 1))

    nc.compile()
    _cache[key] = nc
    return nc


def _scatter_dense(values: np.ndarray, col_indices: np.ndarray) -> np.ndarray:
    Wd = np.zeros((C, B, R, B), np.float32)  # [c, i, r, o]
    vT = np.ascontiguousarray(values.transpose(0, 1, 3, 2))  # [r, k, i, o]
    for r in range(R):
        np.add.at(Wd[:, :, r, :], (col_indices[r],), vT[r])
    return Wd.reshape(D_IN, D_OUT)


def _run(x, values, bias, col_indices, trace=False):
    x = np.asarray(x, np.float32)
    values = np.asarray(values, np.float32)
    bias = np.asarray(bias, np.float32)
    col_indices = np.asarray(col_indices, np.int32)

    W = _scatter_dense(values, col_indices)
    has_bias = bool(np.any(bias))
    FEAT = D_IN // 128 + (1 if has_bias else 0)

    xT = np.ascontiguousarray(x.T)  # [D_IN, N_TOK]
    if has_bias:
        xT = np.concatenate([xT, np.zeros((128, N_TOK), np.float32)], 0)
        xT[D_IN, :] = 1.0
        W = np.concatenate([W, np.zeros((128, D_OUT), np.float32)], 0)
        W[D_IN, :] = bias

    nk8 = NK8
    kf = nk8 * 128
    e4 = ml_dtypes.float8_e4m3
    bf = ml_dtypes.bfloat16

    # fp8 head of the contraction
    x8 = np.ascontiguousarray(
        (xT[:kf] / SW).reshape(nk8 // 2, 2, 128, N_TOK).transpose(2, 0, 1, 3)
    ).astype(e4)                                # [128, NPR, 2, N_TOK]
    W8 = np.ascontiguousarray(
        (W[:kf] * SW).reshape(nk8 // 2, 2, 128, 16, 256).transpose(3, 2, 0, 1, 4)
    ).astype(e4)                                # [OGB, 128, NPR, 2, 256]

    # bf16 tail
    NBF = FEAT - nk8
    Wt = np.ascontiguousarray(
        W[kf:].reshape(NBF, 128, 16, 256).transpose(2, 0, 1, 3)).astype(bf)
    xTc = xT[kf:].astype(bf)

    nc = _build_program(FEAT, nk8)

    in_maps = []
    for c in range(NCORES):
        m = {
            "xT": np.ascontiguousarray(xTc[:, c * TOK:(c + 1) * TOK]),
            "W": Wt,
        }
        if nk8:
            m["x8"] = np.ascontiguousarray(x8[:, :, :, c * TOK:(c + 1) * TOK])
            m["W8"] = W8
        in_maps.append(m)

    res = run_bass_kernel_spmd(nc, in_maps, list(range(NCORES)), trace=trace)

    y = np.empty((N_TOK, D_OUT), np.float32)
    for c in range(NCORES):
        yT = res.results[c]["yT"].reshape(D_OUT, TOK).astype(np.float32)
        y[c * TOK:(c + 1) * TOK, :] = yT.T
    return y, res


def kernel(x: np.ndarray, values: np.ndarray, bias: np.ndarray,
           col_indices: np.ndarray) -> np.ndarray:
    y, _ = _run(x, values, bias, col_indices)
    if not np.isfinite(y).all() or np.abs(y).max() > 1e6:
        y, _ = _run(x, values, bias, col_indices)
    return y


def run_traced(x, values, bias, col_indices):
    return _run(x, values, bias, col_indices, trace=True)[1]


# revision 31
# speedup vs baseline: 1.0197x; 1.0052x over previous
"""Block-ELL sparse linear on 8 Trainium2 cores — bf16 + fp8-DoubleRow hybrid.

Same dense-ified data-parallel strategy as v2, but NK8 of the 32
contraction k-tiles run as fp8-e4m3 DoubleRow matmuls (2 k-tiles per MM at
~0.5 cycles/row) and accumulate into the same PSUM group as the bf16
k-tiles. Weight/x fp8 scales (x/S, W*S) cancel in the product, so no
descale pass is needed. S=8 keeps the small W values (std 0.02) out of
e4m3 subnormals. Measured end-to-end rel err (exact bit-level host-side
simulation over the full output): 1.67e-2 vs the 2e-2 budget.

DR quads are interleaved between the last bf16 quads so the 256-column
(no-FWL) DR LDWEIGHTS hide behind bf16 matmul streams.
"""

import numpy as np
import ml_dtypes
from contextlib import ExitStack

import concourse.bass as bass
import concourse.bacc as bacc
import concourse.tile as tile
from concourse import mybir
from concourse.bass_utils import run_bass_kernel_spmd

N_TOK = 8192
R = 256
C = 256
K = 64
B = 16
D_IN = C * B   # 4096
D_OUT = R * B  # 4096
NCORES = 8
TOK = N_TOK // NCORES  # 1024
TT = 2
TOKT = TOK // TT       # 512

NK8 = 6          # k-tiles (of 32) on the fp8 DoubleRow path; must be even.
                 # 8 would be ~10us faster (measured 417.5us) but its
                 # scale-relative max err is 2.47e-2 > the 2e-2 gate; 6 keeps
                 # both metrics under (L2 1.667e-2, max 1.891e-2).
SW = 8.0         # fp8 weight scale (x gets 1/SW)

_cache = {}


def _build_program(feat_tiles: int, nk8: int):
    key = (feat_tiles, nk8)
    if key in _cache:
        return _cache[key]

    bf16 = mybir.dt.bfloat16
    f32 = mybir.dt.float32
    f8 = mybir.dt.float8e4
    DR = mybir.MatmulPerfMode.DoubleRow

    FEAT = feat_tiles          # total contraction tiles of 128
    NBF = FEAT - nk8           # bf16 k-tiles
    NPR = nk8 // 2             # fp8 DR pairs
    OGB = 16
    OG2 = 2

    nc = bacc.Bacc("TRN2", target_bir_lowering=False, debug=False,
                   num_devices=NCORES)

    xT_d = nc.dram_tensor("xT", [NBF * 128, TOK], bf16, kind="ExternalInput").ap()
    w_d = nc.dram_tensor("W", [OGB, NBF, 128, 256], bf16, kind="ExternalInput").ap()
    if NPR:
        x8_d = nc.dram_tensor("x8", [128, NPR, 2, TOK], f8,
                              kind="ExternalInput").ap()
        w8_d = nc.dram_tensor("W8", [OGB, 128, NPR, 2, 256], f8,
                              kind="ExternalInput").ap()
    yT_d = nc.dram_tensor("yT", [D_OUT // 128, 128, TOK], bf16,
                          kind="ExternalOutput").ap()

    with tile.TileContext(nc) as tc, ExitStack() as ctx:
        xpool = ctx.enter_context(tc.tile_pool(name="x", bufs=1))
        # 6 bufs of f-pair (128KB) tiles = same prefetch byte-depth as the
        # previous 12x64KB, at half the DMA issue count
        wpool = ctx.enter_context(tc.tile_pool(name="w", bufs=6))

        w8pool = ctx.enter_context(tc.tile_pool(name="w8", bufs=3))
        ppool = ctx.enter_context(tc.tile_pool(name="ps", bufs=2, space="PSUM"))
        ypool = ctx.enter_context(tc.tile_pool(name="y", bufs=3))

        # resident bf16 x^T [128, NBF*TOK]
        xt = xpool.tile([128, NBF * TOK], bf16)
        xT_r = xT_d.rearrange("(f p) n -> f p n", p=128)
        for f in range(NBF):
            if f == 0:
                # chunk 0 lands in two halves: the t=0 matmuls can start
                # ~1.3us earlier on the first half while the second streams
                for h in range(TT):
                    nc.sync.dma_start(
                        out=xt[:, h * TOKT:(h + 1) * TOKT],
                        in_=xT_r[0][:, h * TOKT:(h + 1) * TOKT])
            else:
                nc.sync.dma_start(out=xt[:, f * TOK:(f + 1) * TOK], in_=xT_r[f])
        if NPR:
            # resident fp8 x (needed only from the end of the first pass on,
            # so it queues after the bf16 chunks)
            x8t = xpool.tile([128, NPR * 2 * TOK], f8)
            nc.sync.dma_start(out=x8t[:], in_=x8_d.rearrange("p j i n -> p (j i n)"))
            x8v = x8t.rearrange("p (j i n) -> p j i n", j=NPR, i=2)

        def bf_quad(ps, wt, f, first):
            for og2 in range(OG2):
                for t in range(TT):
                    nc.tensor.matmul(
                        ps[og2 * TT + t][:],
                        wt[:, og2 * 128:(og2 + 1) * 128],
                        xt[:, f * TOK + t * TOKT: f * TOK + (t + 1) * TOKT],
                        start=first, stop=False,
                    )

        def dr_quad(ps, w8v, j, last):
            for og2 in range(OG2):
                for t in range(TT):
                    nc.tensor.matmul(
                        ps[og2 * TT + t][:],
                        w8v[:, j, :, og2 * 128:(og2 + 1) * 128],
                        x8v[:, j, :, t * TOKT:(t + 1) * TOKT],
                        start=False, stop=last,
                        perf_mode=DR,
                    )

        def og_pass(ps, ogb, wts=None):
            """One out-group's full contraction: NBF bf16 quads with the DR
            quads woven into the tail so DR LDWEIGHTS overlap bf16 streams."""
            if NPR:
                w8t = w8pool.tile([128, NPR * 2 * 256], f8,
                                  name=f"w8t_{ogb}", tag="w8t")
                nc.scalar.dma_start(out=w8t[:],
                                    in_=w8_d[ogb].rearrange("p j i m -> p (j i m)"))
                w8v = w8t.rearrange("p (j i m) -> p j i m", j=NPR, i=2)
            else:
                w8v = None
            # schedule: bf16 f = 0..NBF-1 with a DR quad after each of the
            # last NPR bf16 quads. W arrives as f-pairs: one 128KB DMA per
            # two k-tiles (same prefetch bytes as before at half the bufs,
            # but half the issue count and supply-latency chain).
            dr_after = {NBF - NPR + j: j for j in range(NPR)}
            for fp in range(NBF // 2):
                wt2 = wpool.tile([128, 512], bf16, name=f"wt_{ogb}_{fp}", tag="wt")
                nc.scalar.dma_start(
                    out=wt2[:].rearrange("p (f c) -> p f c", f=2),
                    in_=w_d[ogb, 2 * fp:2 * fp + 2].rearrange("f p c -> p f c"))
                for fi in range(2):
                    f = 2 * fp + fi
                    bf_quad(ps, wt2[:, fi * 256:(fi + 1) * 256], f,
                            first=(f == 0))
                    j = dr_after.get(f)
                    if j is not None:
                        dr_quad(ps, w8v, j, last=(j == NPR - 1))

        def evict(ps, ogb, last):
            for og2 in range(OG2):
                yt = ypool.tile([128, TOK], bf16, name=f"yt_{ogb}_{og2}", tag="yt")
                og = ogb * OG2 + og2
                for t in range(TT):
                    if (og2 * TT + t) % 2 == 0:
                        nc.vector.tensor_copy(yt[:, t * TOKT:(t + 1) * TOKT],
                                              ps[og2 * TT + t][:])
                    else:
                        nc.scalar.copy(yt[:, t * TOKT:(t + 1) * TOKT],
                                       ps[og2 * TT + t][:])
                    eng = nc.scalar if (last and t == 1) else nc.sync
                    eng.dma_start(
                        out=yT_d[og, :, t * TOKT:(t + 1) * TOKT],
                        in_=yt[:, t * TOKT:(t + 1) * TOKT])

        # --- phase 1: ogb 0,1 interleaved per-f (PE chases the x stream)
        ps01 = []
        for g in range(2):
            ps01.append([ppool.tile([128, TOKT], f32, name=f"ps_{g}_{i}",
                                    tag=f"ps{i}") for i in range(4)])
        w8vs = []
        for f in range(NBF):
            # both out-groups' W tiles for this f in one 2-segment DMA;
            # f=0 lands as two separate halves so the very first matmuls
            # (which need only g0's tile) start ~1.5us earlier
            wt2 = wpool.tile([128, 512], bf16, name=f"wt_p1_{f}", tag="wt")
            if f == 0:
                for g in range(2):
                    nc.scalar.dma_start(
                        out=wt2[:, g * 256:(g + 1) * 256], in_=w_d[g, 0])
            else:
                nc.scalar.dma_start(
                    out=wt2[:].rearrange("p (g c) -> p g c", g=2),
                    in_=w_d[0:2, f].rearrange("g p c -> p g c"))
            wts = [wt2[:, 0:256], wt2[:, 256:512]]
            if f == 1 and NPR:
                # fp8 weights for the pair, queued behind the first bf16 tiles
                for g in range(2):
                    w8t = w8pool.tile([128, NPR * 2 * 256], f8,
                                      name=f"w8t_{g}", tag="w8t")
                    nc.scalar.dma_start(out=w8t[:],
                                        in_=w8_d[g].rearrange("p j i m -> p (j i m)"))
                    w8vs.append(w8t.rearrange("p (j i m) -> p j i m", j=NPR, i=2))
            if f == 0:
                # t-major so the first 8 matmuls need only chunk 0's first
                # half (start=True per psum tile is preserved: one f=0 MM each)
                for t in range(TT):
                    for g in range(2):
                        for og2 in range(OG2):
                            nc.tensor.matmul(
                                ps01[g][og2 * TT + t][:],
                                wts[g][:, og2 * 128:(og2 + 1) * 128],
                                xt[:, t * TOKT:(t + 1) * TOKT],
                                start=True, stop=False,
                            )
            else:
                for g in range(2):
                    bf_quad(ps01[g], wts[g], f, first=False)
        # phase-1 DR quads run after the whole bf16 sweep: by then the
        # x8 DMA (queued last on the sync ring, ~45us) has long landed
        for j in range(NPR):
            for g in range(2):
                dr_quad(ps01[g], w8vs[g], j, last=(j == NPR - 1))
        for g in range(2):
            evict(ps01[g], g, last=False)

        # --- phase 2: remaining out-groups, pipelined
        for ogb in range(2, OGB):
            ps = [ppool.tile([128, TOKT], f32, name=f"ps_{ogb}_{i}", tag=f"ps{i}")
                  for i in range(4)]
            og_pass(ps, ogb)
            evict(ps, ogb, last=(ogb == OGB - 1))

    nc.compile()
    _cache[key] = nc
    return nc


def _scatter_dense(values: np.ndarray, col_indices: np.ndarray) -> np.ndarray:
    Wd = np.zeros((C, B, R, B), np.float32)  # [c, i, r, o]
    vT = np.ascontiguousarray(values.transpose(0, 1, 3, 2))  # [r, k, i, o]
    for r in range(R):
        np.add.at(Wd[:, :, r, :], (col_indices[r],), vT[r])
    return Wd.reshape(D_IN, D_OUT)


def _run(x, values, bias, col_indices, trace=False):
    x = np.asarray(x, np.float32)
    values = np.asarray(values, np.float32)
    bias = np.asarray(bias, np.float32)
    col_indices = np.asarray(col_indices, np.int32)

    W = _scatter_dense(values, col_indices)
    has_bias = bool(np.any(bias))
    FEAT = D_IN // 128 + (1 if has_bias else 0)

    xT = np.ascontiguousarray(x.T)  # [D_IN, N_TOK]
    if has_bias:
        xT = np.concatenate([xT, np.zeros((128, N_TOK), np.float32)], 0)
        xT[D_IN, :] = 1.0
        W = np.concatenate([W, np.zeros((128, D_OUT), np.float32)], 0)
        W[D_IN, :] = bias

    nk8 = NK8
    kf = nk8 * 128
    e4 = ml_dtypes.float8_e4m3
    bf = ml_dtypes.bfloat16

    # fp8 head of the contraction
    x8 = np.ascontiguousarray(
        (xT[:kf] / SW).reshape(nk8 // 2, 2, 128, N_TOK).transpose(2, 0, 1, 3)
    ).astype(e4)                                # [128, NPR, 2, N_TOK]
    W8 = np.ascontiguousarray(
        (W[:kf] * SW).reshape(nk8 // 2, 2, 128, 16, 256).transpose(3, 2, 0, 1, 4)
    ).astype(e4)                                # [OGB, 128, NPR, 2, 256]

    # bf16 tail
    NBF = FEAT - nk8
    Wt = np.ascontiguousarray(
        W[kf:].reshape(NBF, 128, 16, 256).transpose(2, 0, 1, 3)).astype(bf)
    xTc = xT[kf:].astype(bf)

    nc = _build_program(FEAT, nk8)

    in_maps = []
    for c in range(NCORES):
        m = {
            "xT": np.ascontiguousarray(xTc[:, c * TOK:(c + 1) * TOK]),
            "W": Wt,
        }
        if nk8:
            m["x8"] = np.ascontiguousarray(x8[:, :, :, c * TOK:(c + 1) * TOK])
            m["W8"] = W8
        in_maps.append(m)

    res = run_bass_kernel_spmd(nc, in_maps, list(range(NCORES)), trace=trace)

    y = np.empty((N_TOK, D_OUT), np.float32)
    for c in range(NCORES):
        yT = res.results[c]["yT"].reshape(D_OUT, TOK).astype(np.float32)
        y[c * TOK:(c + 1) * TOK, :] = yT.T
    return y, res


def kernel(x: np.ndarray, values: np.ndarray, bias: np.ndarray,
           col_indices: np.ndarray) -> np.ndarray:
    y, _ = _run(x, values, bias, col_indices)
    if not np.isfinite(y).all() or np.abs(y).max() > 1e6:
        y, _ = _run(x, values, bias, col_indices)
    return y


def run_traced(x, values, bias, col_indices):
    return _run(x, values, bias, col_indices, trace=True)[1]
